# revision 18
# baseline (speedup 1.0000x reference)
"""Performer (FAVOR+) linear attention on 8 TRN2 NeuronCores — v2.

Sharding: core c handles batch b=c//4 and head group g=c%4 (4 of 16 heads,
as 2 pairs).  Everything SBUF-resident (no DRAM scratch).

Per core:
  A.  Pair-transposed projections q2T/k2T/v2T [128(2h x 64), 2(pair), N]
      bf16, via fp32r matmuls with 512-wide moving operands.
  K.  Per pair/tile: dash_k = c*kT@projT; E = [1 | exp(dash_k)] raw (no
      bias: the per-row factor w_n = exp(-0.5c^2|k|^2) is folded into V',
      and the global e^{-mk} scale cancels in the output except through
      the eps terms, which are scaled by e^{+mk} instead).
      ctxT[65,267] += V'[128,65].T @ E accumulates in PSUM over tiles,
      V' = [w*v | w].  Finalize: mk = max dash (via max E); ctxT += eps *
      e^{mk} * [sv;N] per partition; Cx = ctxT.T in 3 chunks; Cx row 0 :=
      eps * colsum(Cx real rows)  (consumed by qp's ones column).
  Q.  Per tile: dash_q, mq = rowmax, diag via transpose+square+reduce,
      qp = [1 | exp(dash - 0.5c^2 diag - mq)] bf16, transpose, oe =
      qpT.T @ Cx (64 out cols + denominator col), divide, transpose into
      otb.
  P3. y_tile = otb.T @ Wo-pack, streamed out per tile.

All matmuls are bf16 x bf16 with fp32 PSUM accumulation.
"""
import sys
sys.path.insert(0, '/opt/trn_rl_repo')

import numpy as np
import ml_dtypes
import concourse.bass as bass
import concourse.bacc as bacc
import concourse.tile as tile
from concourse import mybir
from concourse.bass_utils import run_bass_kernel_spmd

F32 = mybir.dt.float32
F32R = mybir.dt.float32r
BF16 = mybir.dt.bfloat16
AX = mybir.AxisListType.X
AF = mybir.ActivationFunctionType
OP = mybir.AluOpType

B, N, D = 2, 4096, 1024
H, DH, M = 16, 64, 266
NT = N // 128
NC = N // 512
CN = DH ** -0.25
EPS = 1e-4
MCH = [(0, 128), (128, 128), (256, 11)]   # chunks over the 267-wide E
MCH2 = [(0, 128), (128, 128), (256, 10)]  # chunks over the 266 features
LIMIT = "all"


def build():
    nc = bacc.Bacc("TRN2", target_bir_lowering=False, debug=False)

    xT = nc.dram_tensor("xT", [D, N], BF16, kind="ExternalInput")
    wP = nc.dram_tensor("wP", [128, 3, 2, 8, 128], BF16, kind="ExternalInput")
    woPb = nc.dram_tensor("woPb", [128, 2048], BF16, kind="ExternalInput")
    projc2 = nc.dram_tensor("projc2", [128, M], BF16, kind="ExternalInput")
    identB = nc.dram_tensor("identB", [128, 128], BF16, kind="ExternalInput")
    identF = nc.dram_tensor("identF", [128, 128], F32, kind="ExternalInput")
    svN = nc.dram_tensor("svN", [65, 4], F32, kind="ExternalInput")
    y = nc.dram_tensor("y", [N, D], F32, kind="ExternalOutput")

    with tile.TileContext(nc) as tc:
        with tc.tile_pool(name="const", bufs=1) as cpool, \
             tc.tile_pool(name="big", bufs=1) as big, \
             tc.tile_pool(name="xt", bufs=2) as xtp, \
             tc.tile_pool(name="strm", bufs=4) as strm, \
             tc.tile_pool(name="sml", bufs=4) as sml, \
             tc.tile_pool(name="psA", bufs=2, space="PSUM") as psA, \
             tc.tile_pool(name="psDa", bufs=2, space="PSUM") as psDa, \
             tc.tile_pool(name="psCtx", bufs=1, space="PSUM") as psCtx, \
             tc.tile_pool(name="psS", bufs=2, space="PSUM") as psS:

            # ---- constants ----
            wPs = cpool.tile([128, 3, 2, 8, 128], BF16, tag="wP")
            nc.sync.dma_start(wPs[:], wP.ap())
            woS = cpool.tile([128, 2048], BF16, tag="wo")
            nc.sync.dma_start(woS[:], woPb.ap())
            pjS = cpool.tile([128, M], BF16, tag="pj")
            nc.sync.dma_start(pjS[:], projc2.ap())
            idB = cpool.tile([128, 128], BF16, tag="idB")
            nc.sync.dma_start(idB[:], identB.ap())
            idF = cpool.tile([128, 128], F32, tag="idF")
            nc.sync.dma_start(idF[:], identF.ap())
            svS = cpool.tile([65, 4], F32, tag="sv")
            nc.sync.dma_start(svS[:], svN.ap())

            # ---- persistent tensors ----
            q2T = big.tile([128, 2, N], BF16, tag="q2T")
            k2T = big.tile([128, 2, N], BF16, tag="k2T")
            v2T = big.tile([128, 2, N], BF16, tag="v2T")
            otb = big.tile([128, 2, N], BF16, tag="otb")
            Eb = big.tile([128, 2, NT, M + 1], BF16, tag="Eb")
            Cx = [big.tile([128, 3, 65], BF16, tag=f"cx{p}", name=f"cx{p}")
                  for p in range(2)]

            nc.vector.memset(Eb[:, :, :, 0:1], 1.0)  # ones column for k_cumsum

            qkv_dst = [q2T, k2T, v2T]

            def copy3(t, dst, src):
                if t == 1:
                    nc.scalar.copy(dst, src)
                else:
                    nc.vector.tensor_copy(dst, src)

            def phase_a(cc):
                xt = xtp.tile([128, 8, 512], BF16, tag="xt")
                nc.sync.dma_start(xt[:], xT.ap().rearrange(
                    "(c p) n -> p c n", p=128)[:, :, cc*512:(cc+1)*512])
                for t in range(3):
                    for pr in range(2):
                        acc = psA.tile([128, 512], F32, tag="a512", name="acc")
                        for dch in range(8):
                            nc.tensor.matmul(acc[:],
                                             wPs[:, t, pr, dch, :],
                                             xt[:, dch, :],
                                             start=(dch == 0), stop=(dch == 7))
                        copy3(t, qkv_dst[t][:, pr, cc*512:(cc+1)*512], acc[:])

            def k_tile(pr, j, ctx_ps, scr):
                kn = scr[:, 128:256]                    # [128,128] bf16
                vn = scr[:, 256:384]
                for hh in range(2):
                    pb = hh * 64
                    dk = psDa.tile([128, M], F32, tag="dash")
                    nc.tensor.matmul(dk[:], k2T[pb:pb+64, pr, j*128:(j+1)*128],
                                     pjS[pb:pb+64, :], start=True, stop=True)
                    nc.scalar.activation(Eb[:, hh, j, 1:M+1], dk[:], AF.Exp)
                nc.tensor.transpose(kn, k2T[:, pr, j*128:(j+1)*128], idB[:])
                nc.tensor.transpose(vn, v2T[:, pr, j*128:(j+1)*128], idB[:])
                sq = strm.tile([128, 128], BF16, tag="sq")
                nc.scalar.activation(sq[:], kn, AF.Square)
                dg2 = sml.tile([128, 2], F32, tag="dg2")
                nc.vector.reduce_sum(dg2[:], sq[:].rearrange("p (h e) -> p h e", e=64),
                                     axis=AX)
                w2 = sml.tile([128, 2], F32, tag="w2")
                nc.scalar.activation(w2[:], dg2[:], AF.Exp, scale=-0.5 * CN * CN)
                vt2 = strm.tile([128, 2, 65], BF16, tag="vt")
                nc.vector.tensor_copy(vt2[:, :, 64], w2[:])
                for hh in range(2):
                    nc.scalar.activation(vt2[:, hh, 0:64], vn[:, hh*64:hh*64+64],
                                         AF.Identity, scale=w2[:, hh:hh+1])
                    nc.tensor.matmul(ctx_ps[hh][:], vt2[:, hh, :], Eb[:, hh, j, :],
                                     start=(j == 0), stop=(j == NT - 1))

            def k_finalize(pr, ctx_ps):
                scr = psS.tile([128, 1024], BF16, tag="scr")
                for hh in range(2):
                    h = pr * 2 + hh
                    m1 = sml.tile([128, NT], BF16, tag="m1")
                    nc.vector.reduce_max(m1[:], Eb[:, hh, :, 1:M+1], axis=AX)
                    m2 = sml.tile([128, 1], BF16, tag="m2")
                    nc.vector.reduce_max(m2[:], m1[:], axis=AX)
                    mrow = scr[0:1, 128:256]
                    nc.tensor.transpose(mrow, m2[:], idB[:])
                    emk = sml.tile([1, 1], F32, tag="emk")
                    nc.vector.reduce_max(emk[:], mrow, axis=AX)
                    emkb = sml.tile([65, 1], F32, tag="emkb")
                    nc.gpsimd.partition_broadcast(emkb[:], emk[:])
                    epscol = sml.tile([65, 1], F32, tag="epscol")
                    nc.vector.tensor_mul(epscol[:], emkb[:], svS[:, h:h+1])
                    cts = strm.tile([65, M + 1], BF16, tag="cts")
                    nc.vector.tensor_scalar_add(cts[:], ctx_ps[hh][:], epscol[:])
                    scc = sml.tile([65, 1], F32, tag="scc")
                    nc.vector.reduce_sum(scc[:], cts[:, 1:M+1], axis=AX)
                    scrow = scr[0:1, 0:130].bitcast(F32)        # [1, 65] f32
                    nc.tensor.transpose(scrow, scc[:], idF[0:65, 0:65])
                    for mc, (off, wd) in enumerate(MCH):
                        cxp = scr[0:128, 384:449]               # [128, 65] bf16
                        nc.tensor.transpose(cxp[0:wd, :], cts[:, off:off+wd],
                                            idB[0:65, 0:65])
                        nc.vector.tensor_copy(Cx[hh][0:wd, mc, :], cxp[0:wd, :])
                    nc.scalar.mul(Cx[hh][0:1, 0, :], scrow, EPS)

            def q_tile(pr, j, scrA, scrB):
                qn = scrA[:, 0:128]
                nc.tensor.transpose(qn, q2T[:, pr, j*128:(j+1)*128], idB[:])
                sq = strm.tile([128, 128], BF16, tag="sq")
                nc.scalar.activation(sq[:], qn, AF.Square)
                dg2 = sml.tile([128, 2], F32, tag="dg2")
                nc.vector.reduce_sum(dg2[:], sq[:].rearrange("p (h e) -> p h e", e=64),
                                     axis=AX)
                for hh in range(2):
                    pb = hh * 64
                    scr = scrA if hh == 0 else scrB
                    dq = psDa.tile([128, M], F32, tag="dash")
                    nc.tensor.matmul(dq[:], q2T[pb:pb+64, pr, j*128:(j+1)*128],
                                     pjS[pb:pb+64, :], start=True, stop=True)
                    rmax = sml.tile([128, 1], F32, tag="rmax")
                    nc.vector.reduce_max(rmax[:], dq[:], axis=AX)
                    bias = sml.tile([128, 1], F32, tag="bias")
                    nc.vector.tensor_scalar(bias[:], dg2[:, hh:hh+1],
                                            -0.5 * CN * CN, rmax[:],
                                            op0=OP.mult, op1=OP.subtract)
                    qp = strm.tile([128, M + 1], BF16, tag="qp")
                    nc.scalar.activation(qp[:, 1:M+1], dq[:], AF.Exp,
                                         bias=bias[:], scale=1.0)
                    nc.scalar.activation(qp[:, 0:1], dq[:, 0:1], AF.Identity,
                                         bias=1.0, scale=0.0)
                    qpt_ps = scr[:, 384:768].rearrange("p (c n) -> p c n", n=128)
                    for mc, (off, wd) in enumerate(MCH):
                        nc.tensor.transpose(qpt_ps[0:wd, mc, :],
                                            qp[:, off:off+wd], idB[:])
                    qpt = strm.tile([128, 3, 128], BF16, tag="qpts")
                    if hh == 0:
                        nc.vector.tensor_copy(qpt[:], qpt_ps)
                    else:
                        nc.scalar.copy(qpt[:], qpt_ps)
                    oe = scr[:, 768:898].bitcast(F32)            # [128, 65]
                    for mc, (off, wd) in enumerate(MCH):
                        nc.tensor.matmul(oe, qpt[0:wd, mc, :], Cx[hh][0:wd, mc, :],
                                         start=(mc == 0), stop=(mc == 2))
                    dinv = sml.tile([128, 1], F32, tag="dinv")
                    nc.vector.reciprocal(dinv[:], oe[:, 64:65])
                    osc = strm.tile([128, 64], BF16, tag="osc")
                    nc.vector.tensor_scalar_mul(osc[:], oe[:, 0:64], dinv[:])
                    ot = scrA[pb:pb+64, 0:128]
                    nc.tensor.transpose(ot, osc[:], idB[:])
                    if hh == 0:
                        nc.vector.tensor_copy(otb[pb:pb+64, pr, j*128:(j+1)*128], ot)
                    else:
                        nc.scalar.copy(otb[pb:pb+64, pr, j*128:(j+1)*128], ot)

            def p3_tile(j):
                ys = strm.tile([128, 1024], F32, tag="ys")
                for half in range(2):
                    yp = psA.tile([128, 512], F32, tag="a512", name="yp")
                    for pr in range(2):
                        nc.tensor.matmul(yp[:], otb[:, pr, j*128:(j+1)*128],
                                         woS[:, pr*1024 + half*512:
                                             pr*1024 + half*512 + 512],
                                         start=(pr == 0), stop=(pr == 1))
                    if half == 0:
                        nc.vector.tensor_copy(ys[:, 0:512], yp[:])
                    else:
                        nc.scalar.copy(ys[:, 512:1024], yp[:])
                nc.sync.dma_start(y.ap()[j*128:(j+1)*128, :], ys[:])

            def zero_y():
                zs = strm.tile([128, 1024], F32, tag="ys")
                nc.vector.memset(zs[:], 0.0)
                for j in range(NT):
                    nc.sync.dma_start(y.ap()[j*128:(j+1)*128, :], zs[:])

            # ---------- schedule ----------
            ctx0 = [psCtx.tile([65, M + 1], F32, tag=f"ctx{hh}", name=f"c0_{hh}")
                    for hh in range(2)]
            for cc in range(NC):
                phase_a(cc)
                if LIMIT != "a":
                    for j in range(cc*4, cc*4 + 4):
                        scr = psS.tile([128, 1024], BF16, tag="scr")
                        k_tile(0, j, ctx0, scr)
            if LIMIT == "a":
                zero_y()
            else:
                k_finalize(0, ctx0)
                ctx1 = [psCtx.tile([65, M + 1], F32, tag=f"ctx{hh}", name=f"c1_{hh}")
                        for hh in range(2)]
                for j in range(NT):
                    scrA = psS.tile([128, 1024], BF16, tag="scr", name="scrA")
                    scrB = psS.tile([128, 1024], BF16, tag="scr", name="scrB")
                    q_tile(0, j, scrA, scrB)
                    if LIMIT != "k0":
                        k_tile(1, j, ctx1, scrA)
                if LIMIT == "k0":
                    zero_y()
                else:
                    k_finalize(1, ctx1)
                    for j in range(NT):
                        scrA = psS.tile([128, 1024], BF16, tag="scr", name="scrA")
                        scrB = psS.tile([128, 1024], BF16, tag="scr", name="scrB")
                        q_tile(1, j, scrA, scrB)
                        p3_tile(j)

    nc.compile()
    return nc


_prog = None


def _build_in_maps(inputs):
    return _make_in_maps(**inputs)


def _make_in_maps(x, Wq, Wk, Wv, Wo, bo, proj):
    x = np.asarray(x, np.float32)
    Wq = np.asarray(Wq, np.float32)
    Wk = np.asarray(Wk, np.float32)
    Wv = np.asarray(Wv, np.float32)
    Wo = np.asarray(Wo, np.float32)
    proj = np.asarray(proj, np.float32)
    cp = np.ascontiguousarray(CN * proj.T)                    # [64, 266]
    projc2 = np.concatenate([cp, cp], axis=0).astype(ml_dtypes.bfloat16)
    identB = np.eye(128, dtype=ml_dtypes.bfloat16)
    identF = np.eye(128, dtype=np.float32)
    xTb = [np.ascontiguousarray(x[b].T) for b in range(B)]
    xsum = [x[b].sum(axis=0) for b in range(B)]               # [1024]
    in_maps = []
    for c in range(8):
        b, g = c // 4, c % 4
        rows = slice(g * 256, g * 256 + 256)
        wPm = np.empty([128, 3, 2, 8, 128], np.float32)
        for t, W in enumerate((Wq, Wk, Wv)):
            blk = W[rows]                                     # [256, 1024]
            for pr in range(2):
                wPm[:, t, pr] = (blk[pr*128:(pr+1)*128].T
                                 .reshape(8, 128, 128).transpose(1, 0, 2))
        woT = Wo[:, rows].T                                   # [256, 1024]
        woP = np.concatenate([woT[:128], woT[128:]], axis=1)  # [128, 2048]
        svNm = np.empty([65, 4], np.float32)
        for h in range(4):
            wvh = Wv[g*256 + h*64: g*256 + (h+1)*64]          # [64, 1024]
            svNm[0:64, h] = EPS * (wvh @ xsum[b])
            svNm[64, h] = EPS * N
        in_maps.append({
            "xT": xTb[b].astype(ml_dtypes.bfloat16),
            "wP": np.ascontiguousarray(wPm).astype(ml_dtypes.bfloat16),
            "woPb": np.ascontiguousarray(woP).astype(ml_dtypes.bfloat16),
            "projc2": projc2,
            "identB": identB,
            "identF": identF,
            "svN": svNm,
        })
    return in_maps


def kernel(x, Wq, Wk, Wv, Wo, bo, proj):
    global _prog
    if _prog is None:
        _prog = build()
    in_maps = _make_in_maps(x, Wq, Wk, Wv, Wo, bo, proj)
    res = run_bass_kernel_spmd(_prog, in_maps, core_ids=list(range(8)))
    out = np.zeros((B, N, D), np.float32)
    for c in range(8):
        out[c // 4] += res.results[c]["y"]
    out += np.asarray(bo, np.float32)[None, None, :]
    return out


# revision 19
# speedup vs baseline: 1.1515x; 1.1515x over previous
"""Performer (FAVOR+) linear attention on 8 TRN2 NeuronCores — v2.

Sharding: core c handles batch b=c//4 and head group g=c%4 (4 of 16 heads,
as 2 pairs).  Everything SBUF-resident (no DRAM scratch).

Per core:
  A.  Pair-transposed projections q2T/k2T/v2T [128(2h x 64), 2(pair), N]
      bf16, via fp32r matmuls with 512-wide moving operands.
  K.  Per pair/tile: dash_k = c*kT@projT; E = [1 | exp(dash_k)] raw (no
      bias: the per-row factor w_n = exp(-0.5c^2|k|^2) is folded into V',
      and the global e^{-mk} scale cancels in the output except through
      the eps terms, which are scaled by e^{+mk} instead).
      ctxT[65,267] += V'[128,65].T @ E accumulates in PSUM over tiles,
      V' = [w*v | w].  Finalize: mk = max dash (via max E); ctxT += eps *
      e^{mk} * [sv;N] per partition; Cx = ctxT.T in 3 chunks; Cx row 0 :=
      eps * colsum(Cx real rows)  (consumed by qp's ones column).
  Q.  Per tile: dash_q, mq = rowmax, diag via transpose+square+reduce,
      qp = [1 | exp(dash - 0.5c^2 diag - mq)] bf16, transpose, oe =
      qpT.T @ Cx (64 out cols + denominator col), divide, transpose into
      otb.
  P3. y_tile = otb.T @ Wo-pack, streamed out per tile.

All matmuls are bf16 x bf16 with fp32 PSUM accumulation.
"""
import sys
sys.path.insert(0, '/opt/trn_rl_repo')

import numpy as np
import ml_dtypes
import concourse.bass as bass
import concourse.bacc as bacc
import concourse.tile as tile
from concourse import mybir
from concourse.bass_utils import run_bass_kernel_spmd

F32 = mybir.dt.float32
F32R = mybir.dt.float32r
BF16 = mybir.dt.bfloat16
AX = mybir.AxisListType.X
AF = mybir.ActivationFunctionType
OP = mybir.AluOpType

B, N, D = 2, 4096, 1024
H, DH, M = 16, 64, 266
NT = N // 128
NC = N // 512
CN = DH ** -0.25
EPS = 1e-4
MCH = [(0, 128), (128, 128), (256, 11)]   # chunks over the 267-wide E
MCH2 = [(0, 128), (128, 128), (256, 10)]  # chunks over the 266 features
LIMIT = "all"


def build():
    nc = bacc.Bacc("TRN2", target_bir_lowering=False, debug=False)

    xT = nc.dram_tensor("xT", [D, N], BF16, kind="ExternalInput")
    wP = nc.dram_tensor("wP", [128, 3, 2, 8, 128], BF16, kind="ExternalInput")
    woPb = nc.dram_tensor("woPb", [128, 2048], BF16, kind="ExternalInput")
    projc2 = nc.dram_tensor("projc2", [128, M], BF16, kind="ExternalInput")
    identB = nc.dram_tensor("identB", [128, 128], BF16, kind="ExternalInput")
    identF = nc.dram_tensor("identF", [128, 128], F32, kind="ExternalInput")
    svN = nc.dram_tensor("svN", [65, 4], F32, kind="ExternalInput")
    y = nc.dram_tensor("y", [N, D], F32, kind="ExternalOutput")

    with tile.TileContext(nc) as tc:
        with tc.tile_pool(name="const", bufs=1) as cpool, \
             tc.tile_pool(name="big", bufs=1) as big, \
             tc.tile_pool(name="xt", bufs=2) as xtp, \
             tc.tile_pool(name="strm", bufs=4) as strm, \
             tc.tile_pool(name="sml", bufs=4) as sml, \
             tc.tile_pool(name="psA", bufs=2, space="PSUM") as psA, \
             tc.tile_pool(name="psDa", bufs=2, space="PSUM") as psDa, \
             tc.tile_pool(name="psCtx", bufs=1, space="PSUM") as psCtx, \
             tc.tile_pool(name="psS", bufs=2, space="PSUM") as psS:

            # ---- constants ----
            wPs = cpool.tile([128, 3, 2, 8, 128], BF16, tag="wP")
            nc.sync.dma_start(wPs[:], wP.ap())
            woS = cpool.tile([128, 2048], BF16, tag="wo")
            nc.sync.dma_start(woS[:], woPb.ap())
            pjS = cpool.tile([128, M], BF16, tag="pj")
            nc.sync.dma_start(pjS[:], projc2.ap())
            idB = cpool.tile([128, 128], BF16, tag="idB")
            nc.sync.dma_start(idB[:], identB.ap())
            idF = cpool.tile([128, 128], F32, tag="idF")
            nc.sync.dma_start(idF[:], identF.ap())
            svS = cpool.tile([65, 4], F32, tag="sv")
            nc.sync.dma_start(svS[:], svN.ap())

            # ---- persistent tensors ----
            q2T = big.tile([128, 2, N], BF16, tag="q2T")
            k2T = big.tile([128, 2, N], BF16, tag="k2T")
            v2T = big.tile([128, 2, N], BF16, tag="v2T")
            otb = big.tile([128, 2, N], BF16, tag="otb")
            Eb = big.tile([128, 2, NT, M + 1], BF16, tag="Eb")
            Cx = [big.tile([128, 3, 65], BF16, tag=f"cx{p}", name=f"cx{p}")
                  for p in range(2)]

            nc.vector.memset(Eb[:, :, :, 0:1], 1.0)  # ones column for k_cumsum

            qkv_dst = [q2T, k2T, v2T]

            def copy3(t, dst, src):
                if t == 1:
                    nc.scalar.copy(dst, src)
                else:
                    nc.vector.tensor_copy(dst, src)

            def phase_a(cc):
                xt = xtp.tile([128, 8, 512], BF16, tag="xt")
                nc.sync.dma_start(xt[:], xT.ap().rearrange(
                    "(c p) n -> p c n", p=128)[:, :, cc*512:(cc+1)*512])
                for t in range(3):
                    for pr in range(2):
                        acc = psA.tile([128, 512], F32, tag="a512", name="acc")
                        for dch in range(8):
                            nc.tensor.matmul(acc[:],
                                             wPs[:, t, pr, dch, :],
                                             xt[:, dch, :],
                                             start=(dch == 0), stop=(dch == 7))
                        copy3(t, qkv_dst[t][:, pr, cc*512:(cc+1)*512], acc[:])

            def k_tile(pr, j, ctx_ps, scr):
                kn = scr[:, 128:256]                    # [128,128] bf16
                vn = scr[:, 256:384]
                for hh in range(2):
                    pb = hh * 64
                    dk = psDa.tile([128, M], F32, tag="dash")
                    nc.tensor.matmul(dk[:], k2T[pb:pb+64, pr, j*128:(j+1)*128],
                                     pjS[pb:pb+64, :], start=True, stop=True)
                    nc.scalar.activation(Eb[:, hh, j, 1:M+1], dk[:], AF.Exp)
                nc.tensor.transpose(kn, k2T[:, pr, j*128:(j+1)*128], idB[:])
                nc.tensor.transpose(vn, v2T[:, pr, j*128:(j+1)*128], idB[:])
                sq = strm.tile([128, 128], BF16, tag="sq")
                nc.scalar.activation(sq[:], kn, AF.Square)
                dg2 = sml.tile([128, 2], F32, tag="dg2")
                nc.vector.reduce_sum(dg2[:], sq[:].rearrange("p (h e) -> p h e", e=64),
                                     axis=AX)
                w2 = sml.tile([128, 2], F32, tag="w2")
                nc.scalar.activation(w2[:], dg2[:], AF.Exp, scale=-0.5 * CN * CN)
                vt2 = strm.tile([128, 2, 65], BF16, tag="vt")
                nc.vector.tensor_copy(vt2[:, :, 64], w2[:])
                for hh in range(2):
                    nc.scalar.activation(vt2[:, hh, 0:64], vn[:, hh*64:hh*64+64],
                                         AF.Identity, scale=w2[:, hh:hh+1])
                    nc.tensor.matmul(ctx_ps[hh][:], vt2[:, hh, :], Eb[:, hh, j, :],
                                     start=(j == 0), stop=(j == NT - 1))

            def k_finalize(pr, ctx_ps):
                scr = psS.tile([128, 1024], BF16, tag="scr")
                for hh in range(2):
                    h = pr * 2 + hh
                    m1 = sml.tile([128, NT], BF16, tag="m1")
                    nc.vector.reduce_max(m1[:], Eb[:, hh, :, 1:M+1], axis=AX)
                    m2 = sml.tile([128, 1], BF16, tag="m2")
                    nc.vector.reduce_max(m2[:], m1[:], axis=AX)
                    mrow = scr[0:1, 128:256]
                    nc.tensor.transpose(mrow, m2[:], idB[:])
                    emk = sml.tile([1, 1], F32, tag="emk")
                    nc.vector.reduce_max(emk[:], mrow, axis=AX)
                    emkb = sml.tile([65, 1], F32, tag="emkb")
                    nc.gpsimd.partition_broadcast(emkb[:], emk[:])
                    epscol = sml.tile([65, 1], F32, tag="epscol")
                    nc.vector.tensor_mul(epscol[:], emkb[:], svS[:, h:h+1])
                    cts = strm.tile([65, M + 1], BF16, tag="cts")
                    nc.vector.tensor_scalar_add(cts[:], ctx_ps[hh][:], epscol[:])
                    scc = sml.tile([65, 1], F32, tag="scc")
                    nc.vector.reduce_sum(scc[:], cts[:, 1:M+1], axis=AX)
                    scrow = scr[0:1, 0:130].bitcast(F32)        # [1, 65] f32
                    nc.tensor.transpose(scrow, scc[:], idF[0:65, 0:65])
                    for mc, (off, wd) in enumerate(MCH):
                        cxp = scr[0:128, 384:449]               # [128, 65] bf16
                        nc.tensor.transpose(cxp[0:wd, :], cts[:, off:off+wd],
                                            idB[0:65, 0:65])
                        nc.vector.tensor_copy(Cx[hh][0:wd, mc, :], cxp[0:wd, :])
                    nc.scalar.mul(Cx[hh][0:1, 0, :], scrow, EPS)

            def q_tile(pr, j, scrA, scrB=None):
                scrB = scrA if scrB is None else scrB
                qn = scrA[:, 0:128]
                nc.tensor.transpose(qn, q2T[:, pr, j*128:(j+1)*128], idB[:])
                sq = strm.tile([128, 128], BF16, tag="sq")
                nc.scalar.activation(sq[:], qn, AF.Square)
                dg2 = sml.tile([128, 2], F32, tag="dg2")
                nc.vector.reduce_sum(dg2[:], sq[:].rearrange("p (h e) -> p h e", e=64),
                                     axis=AX)
                for hh in range(2):
                    pb = hh * 64
                    scr = scrA if hh == 0 else scrB
                    dq = psDa.tile([128, M], F32, tag="dash")
                    nc.tensor.matmul(dq[:], q2T[pb:pb+64, pr, j*128:(j+1)*128],
                                     pjS[pb:pb+64, :], start=True, stop=True)
                    rmax = sml.tile([128, 1], F32, tag="rmax")
                    nc.vector.reduce_max(rmax[:], dq[:], axis=AX)
                    bias = sml.tile([128, 1], F32, tag="bias")
                    nc.vector.tensor_scalar(bias[:], dg2[:, hh:hh+1],
                                            -0.5 * CN * CN, rmax[:],
                                            op0=OP.mult, op1=OP.subtract)
                    qp = strm.tile([128, M + 1], BF16, tag="qp")
                    nc.scalar.activation(qp[:, 1:M+1], dq[:], AF.Exp,
                                         bias=bias[:], scale=1.0)
                    nc.gpsimd.memset(qp[:, 0:1], 1.0)
                    qpt_ps = scr[:, 384:768].rearrange("p (c n) -> p c n", n=128)
                    for mc, (off, wd) in enumerate(MCH):
                        nc.tensor.transpose(qpt_ps[0:wd, mc, :],
                                            qp[:, off:off+wd], idB[:])
                    qpt = strm.tile([128, 3, 128], BF16, tag="qpts")
                    if hh == 0:
                        nc.vector.tensor_copy(qpt[:], qpt_ps)
                    else:
                        nc.scalar.copy(qpt[:], qpt_ps)
                    oe = scr[:, 768:898].bitcast(F32)            # [128, 65]
                    for mc, (off, wd) in enumerate(MCH):
                        nc.tensor.matmul(oe, qpt[0:wd, mc, :], Cx[hh][0:wd, mc, :],
                                         start=(mc == 0), stop=(mc == 2))
                    dinv = sml.tile([128, 1], F32, tag="dinv")
                    nc.vector.reciprocal(dinv[:], oe[:, 64:65])
                    osc = strm.tile([128, 64], BF16, tag="osc")
                    nc.vector.tensor_scalar_mul(osc[:], oe[:, 0:64], dinv[:])
                    ot = scrA[pb:pb+64, 0:128]
                    nc.tensor.transpose(ot, osc[:], idB[:])
                    if hh == 0:
                        nc.vector.tensor_copy(otb[pb:pb+64, pr, j*128:(j+1)*128], ot)
                    else:
                        nc.scalar.copy(otb[pb:pb+64, pr, j*128:(j+1)*128], ot)

            def p3_tile(j):
                ys = strm.tile([128, 1024], F32, tag="ys")
                for half in range(2):
                    yp = psA.tile([128, 512], F32, tag="a512", name="yp")
                    for pr in range(2):
                        nc.tensor.matmul(yp[:], otb[:, pr, j*128:(j+1)*128],
                                         woS[:, pr*1024 + half*512:
                                             pr*1024 + half*512 + 512],
                                         start=(pr == 0), stop=(pr == 1))
                    if half == 0:
                        nc.vector.tensor_copy(ys[:, 0:512], yp[:])
                    else:
                        nc.scalar.copy(ys[:, 512:1024], yp[:])
                nc.sync.dma_start(y.ap()[j*128:(j+1)*128, :], ys[:])

            def zero_y():
                zs = strm.tile([128, 1024], F32, tag="ys")
                nc.vector.memset(zs[:], 0.0)
                for j in range(NT):
                    nc.sync.dma_start(y.ap()[j*128:(j+1)*128, :], zs[:])

            # ---------- schedule ----------
            ctx0 = [psCtx.tile([65, M + 1], F32, tag=f"ctx{hh}", name=f"c0_{hh}")
                    for hh in range(2)]
            for cc in range(NC):
                phase_a(cc)
                if LIMIT != "a":
                    for j in range(cc*4, cc*4 + 4):
                        scr = psS.tile([128, 1024], BF16, tag="scr")
                        k_tile(0, j, ctx0, scr)
            if LIMIT == "a":
                zero_y()
            else:
                k_finalize(0, ctx0)
                ctx1 = [psCtx.tile([65, M + 1], F32, tag=f"ctx{hh}", name=f"c1_{hh}")
                        for hh in range(2)]
                for j in range(NT):
                    scrA = psS.tile([128, 1024], BF16, tag="scr", name="scrA")
                    q_tile(0, j, scrA)
                    if LIMIT != "k0":
                        k_tile(1, j, ctx1, scrA)
                if LIMIT == "k0":
                    zero_y()
                else:
                    k_finalize(1, ctx1)
                    for j in range(NT):
                        scrA = psS.tile([128, 1024], BF16, tag="scr", name="scrA")
                        q_tile(1, j, scrA)
                        p3_tile(j)

    nc.compile()
    return nc


_prog = None


def _build_in_maps(inputs):
    return _make_in_maps(**inputs)


def _make_in_maps(x, Wq, Wk, Wv, Wo, bo, proj):
    x = np.asarray(x, np.float32)
    Wq = np.asarray(Wq, np.float32)
    Wk = np.asarray(Wk, np.float32)
    Wv = np.asarray(Wv, np.float32)
    Wo = np.asarray(Wo, np.float32)
    proj = np.asarray(proj, np.float32)
    cp = np.ascontiguousarray(CN * proj.T)                    # [64, 266]
    projc2 = np.concatenate([cp, cp], axis=0).astype(ml_dtypes.bfloat16)
    identB = np.eye(128, dtype=ml_dtypes.bfloat16)
    identF = np.eye(128, dtype=np.float32)
    xTb = [np.ascontiguousarray(x[b].T) for b in range(B)]
    xsum = [x[b].sum(axis=0) for b in range(B)]               # [1024]
    in_maps = []
    for c in range(8):
        b, g = c // 4, c % 4
        rows = slice(g * 256, g * 256 + 256)
        wPm = np.empty([128, 3, 2, 8, 128], np.float32)
        for t, W in enumerate((Wq, Wk, Wv)):
            blk = W[rows]                                     # [256, 1024]
            for pr in range(2):
                wPm[:, t, pr] = (blk[pr*128:(pr+1)*128].T
                                 .reshape(8, 128, 128).transpose(1, 0, 2))
        woT = Wo[:, rows].T                                   # [256, 1024]
        woP = np.concatenate([woT[:128], woT[128:]], axis=1)  # [128, 2048]
        svNm = np.empty([65, 4], np.float32)
        for h in range(4):
            wvh = Wv[g*256 + h*64: g*256 + (h+1)*64]          # [64, 1024]
            svNm[0:64, h] = EPS * (wvh @ xsum[b])
            svNm[64, h] = EPS * N
        in_maps.append({
            "xT": xTb[b].astype(ml_dtypes.bfloat16),
            "wP": np.ascontiguousarray(wPm).astype(ml_dtypes.bfloat16),
            "woPb": np.ascontiguousarray(woP).astype(ml_dtypes.bfloat16),
            "projc2": projc2,
            "identB": identB,
            "identF": identF,
            "svN": svNm,
        })
    return in_maps


def kernel(x, Wq, Wk, Wv, Wo, bo, proj):
    global _prog
    if _prog is None:
        _prog = build()
    in_maps = _make_in_maps(x, Wq, Wk, Wv, Wo, bo, proj)
    res = run_bass_kernel_spmd(_prog, in_maps, core_ids=list(range(8)))
    out = np.zeros((B, N, D), np.float32)
    for c in range(8):
        out[c // 4] += res.results[c]["y"]
    out += np.asarray(bo, np.float32)[None, None, :]
    return out
